# revision 1
# baseline (speedup 1.0000x reference)
"""Contrastive loss (soft-target NT-Xent style) on 8 Trainium2 NeuronCores.

Math (matches the reference):
    e = x / max(||x||, eps)              row-normalized embeddings
    sim = e @ e.T / T                    T = 0.1
    logz_i = logsumexp_{j != i} sim[i, j]
    row_loss_i = sum_{j: l_j == l_i, j != i} (logz_i - sim[i, j])
    loss = sum_i row_loss_i / N

Host prep: rows are L2-normalized and scaled by sqrt(C16) with
C16 = 10*log2(e)*2^16, so on-device psum sim values equal g*C16 where
g = e_i.e_j.  Each core gets its inputs rotated by c*1024 along the sample
axis so all 8 cores run the identical program on "local rows 0..1023" and
every core's diagonal lands in local column group 0.

Device decomposition (per core):
    sumexp_i = sum_j exp(10*g_ij - 10) - 1       (diag g_ii ~= 1 exactly)
    logz_i   = 10 + ln(sumexp_i)
    d_i      = e_i . S_{l_i},  S_k = sum_{j: l_j = k} e_j   (via 2 matmuls)
    row_loss_i = C_{l_i} * logzf_i - 10*d_i - lnres_i
    (logzf = 10 + lnres, lnres = ln(sumexp))
The 32 exp units (2048 cols each) are split between ScalarE (exact exp,
fused accumulate) and VectorE: the vector units compute Schraudolph
float-bits int32(P*128 + B) via the f32->i32 output convert, whose bitcast
IS 2^w ~= exp(10g-10), then reduce the bitcast-f32 view.  Group 0 (holding
the diagonal) always stays on ScalarE so the exact -1 diag subtraction
survives.  Each core returns the scalar sum of its 1024 row losses; host
sums and divides by N.
"""

import math

import numpy as np

import concourse.bass as bass
import concourse.bacc as bacc
import concourse.tile as tile
from concourse import mybir

N = 8192
D = 128
NCLASS = 100
NCORES = 8
ROWS = N // NCORES  # rows per core (1024)
MT = ROWS // 128  # m-tiles per core (8)
CH = N // 128  # 128-row chunks (64)
NS = 8  # column slices (each 1024 wide)
SW = N // NS  # slice width (1024)
NG = 4  # psum groups per m-tile (each 2048 wide)
TEMP_INV = 10.0  # 1 / temperature

LOG2E = math.log2(math.e)
C16 = TEMP_INV * LOG2E * 2.0**16  # embedding scale^2 folded in on host
ACT_SCALE = math.log(2.0) / 2.0**16  # psum -> exp argument (with bias -10)
SCH_SIGMA = 0.0566  # Schraudolph mean-bias correction
SCH_B = (127.0 - SCH_SIGMA - TEMP_INV * LOG2E) * 2.0**23 - 16.0
SCH_K = 128.0  # 2^23 / 2^16
# int16/bf16 variant: bits16 = w*2^7 + (127-sigma-10*log2e)*2^7
SCH_B16 = (127.0 - SCH_SIGMA - TEMP_INV * LOG2E) * 2.0**7
SCH_K16 = 2.0**-9  # 2^7 / 2^16

F32 = mybir.dt.float32
BF16 = mybir.dt.bfloat16
I32 = mybir.dt.int32
I16 = mybir.dt.int16


def build_nc(loop_k: int = 1, stage: int = 4, n_dve: int = 22, ed_bufs: int = 2, ib_bufs: int = 2, sch16: int = 1):
    """Build the per-core Bass program. loop_k > 1 wraps the whole body in a
    hardware loop (timing amortization only). stage < 4 builds a prefix.

    n_dve of the 32 exp units run on VectorE via the Schraudolph bits trick;
    group 0 of each m-tile (holds the diagonal) always stays on ScalarE."""
    base, extra = divmod(n_dve, MT)
    sdve = [min(3, base + (1 if m < extra else 0)) for m in range(MT)]
    nc = bacc.Bacc("TRN2", target_bir_lowering=False, debug=False)

    xt_d = nc.dram_tensor("xt", [128, N], BF16, kind="ExternalInput")
    xrm_d = nc.dram_tensor("xrm", [128, CH * 128], BF16, kind="ExternalInput")
    ohb_d = nc.dram_tensor("ohb", [128, CH * NCLASS], BF16, kind="ExternalInput")
    oh8_d = nc.dram_tensor("oh8", [128, MT * NCLASS], F32, kind="ExternalInput")
    cb_d = nc.dram_tensor("cb", [128, NCLASS], F32, kind="ExternalInput")
    out_d = nc.dram_tensor("out", [1, 1], F32, kind="ExternalOutput")

    with tile.TileContext(nc) as tc:
        with (
            tc.tile_pool(name="persist", bufs=1) as persist,
            tc.tile_pool(name="edum", bufs=ed_bufs) as edum_pool,
            tc.tile_pool(name="ibp", bufs=ib_bufs) as ib_pool,
        ):
            # ---- persistent SBUF tiles ----
            etn_s = [persist.tile([128, SW], BF16, tag=f"etn{i}", name=f"etn{i}") for i in range(NS)]
            erm = persist.tile([128, CH, 128], BF16, tag="erm")
            ohb = persist.tile([128, CH, NCLASS], BF16, tag="ohb")
            oh8 = persist.tile([128, MT, NCLASS], F32, tag="oh8")
            cb = persist.tile([128, NCLASS], F32, tag="cb")
            stsb = persist.tile([128, NCLASS], BF16, tag="stsb")
            t10 = persist.tile([128, MT, NCLASS], F32, tag="t10")
            expacc = persist.tile([128, MT * NG], F32, tag="expacc")
            sum4 = persist.tile([128, MT], F32, tag="sum4")
            lnres = persist.tile([128, MT], F32, tag="lnres")
            logzf = persist.tile([128, MT], F32, tag="logzf")
            rl = persist.tile([128, MT], F32, tag="rl")
            am = persist.tile([128, MT], F32, tag="am")
            bm = persist.tile([128, MT], F32, tag="bm")
            rlrow = persist.tile([128, 1], F32, tag="rlrow")
            ones = persist.tile([128, 1], F32, tag="ones")
            u0 = persist.tile([128, NCLASS], F32, tag="u0")
            u1 = persist.tile([128, NCLASS], F32, tag="u1")
            outsb = persist.tile([1, 1], F32, tag="outsb")
            bneg10 = persist.tile([128, 1], F32, tag="bneg10")
            bneg1 = persist.tile([128, 1], F32, tag="bneg1")
            dps = persist.tile([128, 3], F32, tag="dps")
            dtot = persist.tile([128, 1], F32, tag="dtot")
            sum4a = persist.tile([128, 1], F32, tag="sum4a")

            nc.vector.memset(ones[:], 1.0)
            nc.vector.memset(bneg10[:], -TEMP_INV)
            nc.vector.memset(bneg1[:], -1.0)

            xrmv = xrm_d.rearrange("p (c k) -> p c k", k=128)
            ohbv = ohb_d.rearrange("p (c k) -> p c k", k=NCLASS)
            oh8v = oh8_d.rearrange("p (m k) -> p m k", k=NCLASS)

            def finish(src):
                nc.vector.tensor_reduce(
                    out=rlrow[:],
                    in_=src,
                    axis=mybir.AxisListType.X,
                    op=mybir.AluOpType.add,
                )
                with tc.tile_pool(name="fpsum", bufs=1, space="PSUM") as fpsum:
                    fin = fpsum.tile([1, 1], F32, tag="fin")
                    nc.tensor.matmul(fin[:], rlrow[:], ones[:], start=True, stop=True)
                    nc.vector.tensor_copy(outsb[:], fin[:])
                nc.sync.dma_start(out_d[:], outsb[:])

            def body():
                # ---- input DMA: main-loop feeds first, tail feeds after ----
                for i in range(NS):
                    nc.sync.dma_start(etn_s[i][:], xt_d[:, i * SW : (i + 1) * SW])
                for gq in range(4):
                    s = slice(gq * (CH // 4), (gq + 1) * (CH // 4))
                    nc.sync.dma_start(erm[:, s, :], xrmv[:, s, :])
                for gq in range(4):
                    s = slice(gq * (CH // 4), (gq + 1) * (CH // 4))
                    nc.sync.dma_start(ohb[:, s, :], ohbv[:, s, :])
                nc.sync.dma_start(oh8[:], oh8v[:])
                nc.sync.dma_start(cb[:], cb_d[:])

                if stage == 1:
                    finish(etn_s[0][:, :64])
                    return

                # ---- main loop: sim blocks, exp units split ScalarE/VectorE
                with tc.tile_pool(name="mpsum", bufs=2, space="PSUM") as mpsum:
                    for m in range(MT):
                        lhsT = etn_s[0][:, m * 128 : (m + 1) * 128]
                        sm = sdve[m]
                        na = NG - sm
                        ib = (
                            ib_pool.tile([128, 3, 2048], I16 if sch16 else I32, tag="ib", name="ib")
                            if sm
                            else None
                        )
                        for g in range(NG):
                            ps = mpsum.tile([128, 2048], F32, tag="ps")
                            for q in range(4):
                                n0 = (g * 4 + q) * 512
                                nc.tensor.matmul(
                                    ps[:, q * 512 : (q + 1) * 512],
                                    lhsT,
                                    etn_s[n0 // SW][:, n0 % SW : n0 % SW + 512],
                                    start=True,
                                    stop=True,
                                )
                            if g >= na:
                                nc.vector.tensor_scalar(
                                    out=ib[:, g - na, :],
                                    in0=ps[:],
                                    scalar1=SCH_K16 if sch16 else SCH_K,
                                    scalar2=SCH_B16 if sch16 else SCH_B,
                                    op0=mybir.AluOpType.mult,
                                    op1=mybir.AluOpType.add,
                                )
                            else:
                                ed = edum_pool.tile([128, 2048], BF16, tag="ed")
                                nc.scalar.activation(
                                    ed[:],
                                    ps[:],
                                    mybir.ActivationFunctionType.Exp,
                                    bias=bneg10[:],
                                    scale=ACT_SCALE,
                                    accum_out=expacc[
                                        :, m * NG + g : m * NG + g + 1
                                    ],
                                )
                        if sm:
                            nc.vector.tensor_reduce(
                                out=dps[:, :sm],
                                in_=ib[:, 0:sm, :].bitcast(BF16 if sch16 else F32),
                                axis=mybir.AxisListType.X,
                                op=mybir.AluOpType.add,
                            )
                            nc.vector.tensor_reduce(
                                out=sum4a[:],
                                in_=expacc[:, m * NG : m * NG + na],
                                axis=mybir.AxisListType.X,
                                op=mybir.AluOpType.add,
                            )
                            if sm == 1:
                                nc.vector.tensor_add(
                                    sum4[:, m : m + 1], sum4a[:], dps[:, 0:1]
                                )
                            else:
                                nc.vector.tensor_reduce(
                                    out=dtot[:],
                                    in_=dps[:, :sm],
                                    axis=mybir.AxisListType.X,
                                    op=mybir.AluOpType.add,
                                )
                                nc.vector.tensor_add(
                                    sum4[:, m : m + 1], sum4a[:], dtot[:]
                                )
                        else:
                            nc.vector.tensor_reduce(
                                out=sum4[:, m : m + 1],
                                in_=expacc[:, m * NG : (m + 1) * NG],
                                axis=mybir.AxisListType.X,
                                op=mybir.AluOpType.add,
                            )

                if stage == 2:
                    finish(sum4[:])
                    return

                # ---- tail: class sums ST, per-row positive dots, row losses.
                # PE runs this after the sim matmuls; ScalarE/VectorE overlap
                # their remaining exp units with it.
                with tc.tile_pool(name="tpsum", bufs=1, space="PSUM") as tpsum:
                    st_ps = tpsum.tile([128, NCLASS], F32, tag="st")
                    for c in range(CH):
                        nc.tensor.matmul(
                            st_ps[:],
                            erm[:, c, :],
                            ohb[:, c, :],
                            start=(c == 0),
                            stop=(c == CH - 1),
                        )
                    nc.vector.tensor_copy(stsb[:], st_ps[:])
                    for m in range(MT):
                        tm_ps = tpsum.tile([128, NCLASS], F32, tag="tm", bufs=2)
                        nc.tensor.matmul(
                            tm_ps[:],
                            etn_s[0][:, m * 128 : (m + 1) * 128],
                            stsb[:],
                            start=True,
                            stop=True,
                        )
                        nc.vector.tensor_scalar_mul(t10[:, m, :], tm_ps[:], ACT_SCALE)

                for m in range(MT):
                    nc.vector.scalar_tensor_tensor(
                        out=u0[:],
                        in0=oh8[:, m, :],
                        scalar=1.0,
                        in1=cb[:],
                        op0=mybir.AluOpType.mult,
                        op1=mybir.AluOpType.mult,
                        accum_out=am[:, m : m + 1],
                    )
                    nc.vector.scalar_tensor_tensor(
                        out=u1[:],
                        in0=oh8[:, m, :],
                        scalar=1.0,
                        in1=t10[:, m, :],
                        op0=mybir.AluOpType.mult,
                        op1=mybir.AluOpType.mult,
                        accum_out=bm[:, m : m + 1],
                    )

                if stage == 3:
                    finish(t10[:, :, :].rearrange("p a b -> p (a b)"))
                    return

                # ---- row losses ----
                nc.scalar.activation(
                    lnres[:], sum4[:], mybir.ActivationFunctionType.Ln, bias=bneg1[:]
                )
                nc.vector.tensor_scalar_add(logzf[:], lnres[:], TEMP_INV)
                # row_loss = am*logzf - bm - lnres
                nc.vector.tensor_mul(rl[:], am[:], logzf[:])
                nc.vector.tensor_sub(rl[:], rl[:], bm[:])
                nc.vector.tensor_sub(rl[:], rl[:], lnres[:])
                finish(rl[:])

            if loop_k == 1:
                body()
            else:
                with tc.For_i(0, loop_k, 1):
                    body()

    nc.compile()
    return nc


def prepare_inputs(embeddings: np.ndarray, labels: np.ndarray):
    """Host-side shard prep: normalize+scale rows, per-core rotated views
    (transposed and row-major) + label one-hots."""
    import ml_dtypes

    x = np.asarray(embeddings, dtype=np.float64)
    rn = 1.0 / np.maximum(np.sqrt((x * x).sum(axis=1)), 1e-12)
    xn = (x * (rn * math.sqrt(C16))[:, None]).astype(ml_dtypes.bfloat16)
    lab = np.asarray(labels).astype(np.int64).ravel()
    counts = np.bincount(lab, minlength=NCLASS).astype(np.float32)
    cb_host = np.ascontiguousarray(np.broadcast_to(counts[None, :], (128, NCLASS)))
    in_maps = []
    for c in range(NCORES):
        perm = np.roll(np.arange(N), -c * ROWS)
        xr = xn[perm]  # [N, D] rotated, normalized, scaled
        xt_host = np.ascontiguousarray(xr.T)  # [128, N]
        xrm_host = np.ascontiguousarray(
            xr.reshape(CH, 128, D).transpose(1, 0, 2).reshape(128, -1)
        )
        oh = (lab[perm, None] == np.arange(NCLASS)[None, :]).astype(np.float32)
        oh_pck = oh.reshape(CH, 128, NCLASS).transpose(1, 0, 2)
        ohb_host = np.ascontiguousarray(
            oh_pck.reshape(128, -1).astype(ml_dtypes.bfloat16)
        )
        oh8_host = np.ascontiguousarray(oh_pck[:, :MT, :].reshape(128, -1))
        in_maps.append(
            {
                "xt": xt_host,
                "xrm": xrm_host,
                "ohb": ohb_host,
                "oh8": oh8_host,
                "cb": cb_host,
            }
        )
    return in_maps


_NC_CACHE = {}


def kernel(embeddings: np.ndarray, labels: np.ndarray) -> np.ndarray:
    from concourse.bass_utils import run_bass_kernel_spmd

    nc = _NC_CACHE.get("nc")
    if nc is None:
        nc = _NC_CACHE["nc"] = build_nc(loop_k=1)
    in_maps = prepare_inputs(embeddings, labels)
    res = run_bass_kernel_spmd(nc, in_maps, list(range(NCORES)))
    total = sum(float(r["out"][0, 0]) for r in res.results)
    return np.asarray(total / N, dtype=np.float32)



# revision 2
# speedup vs baseline: 4.0384x; 4.0384x over previous
"""Contrastive loss (soft-target NT-Xent) on 8 Trainium2 NeuronCores.

Math (matches the reference):
    e = x / max(||x||, eps)              row-normalized embeddings
    sim = e @ e.T / T                    T = 0.1
    logz_i = logsumexp_{j != i} sim[i, j]
    row_loss_i = sum_{j: l_j == l_i, j != i} (logz_i - sim[i, j])
    loss = sum_i row_loss_i / N

Decomposition: the device only computes the exp-sums; everything involving
labels / positive pairs / logs runs on the host.

Each unordered pair {i, j} is exp'd ONCE using a wrapped band: chunk c1
(128 rows) covers chunk-columns c1..c1+32 (mod 64).  Row-sums of an exp'd
block feed sumexp for its rows (free-axis accumulate); column-sums feed
sumexp for its columns (partition reduce via a ones^T @ exptile matmul
accumulated in PSUM).  The diagonal chunk (both orders present in the
block) and the antipodal chunk (distance 32, computed by both endpoints)
contribute row-sums only.  The band is shift invariant, so all 8 cores run
the identical program on inputs rotated by c*1024 samples; each core needs
only local columns [0, 5120).

Per core the device returns row-sum partials rowp [128, NSLOT] (one slot
per exp piece) and column-sum partials colp [1, 5120] (garbage outside
[128, 4992), ignored by the host).  Host: sumexp_i = rowparts + colparts -
exp(10*g_ii - 10) (exact diag from the quantized embeddings), logz = 10 +
ln(sumexp), row_loss = C_i*logz - 10*(d_i - 1) with d_i = e_i . S_{l_i}.

Exp pieces are split between ScalarE (exact exp, fused row accumulate) and
VectorE (Schraudolph float-bits int16 trick + 4x bf16 reduce), ratio
tunable via n_sc.
"""

import math

import numpy as np

import concourse.bass as bass
import concourse.bacc as bacc
import concourse.tile as tile
from concourse import mybir

N = 8192
D = 128
NCLASS = 100
NCORES = 8
CH = 64  # 128-row chunks
BAND = 33  # chunks per strip (diag + 32)
LCOLS = 7 * 128 + BAND * 128  # local column span = 5120
CSPAN_LO, CSPAN_HI = 128, 7 * 128 + 4096  # useful colsum range
TEMP_INV = 10.0

LOG2E = math.log2(math.e)
C16 = TEMP_INV * LOG2E * 2.0**16  # embedding scale^2 folded in on host
ACT_SCALE = math.log(2.0) / 2.0**16  # psum -> exp argument (with bias -10)
SCH_SIGMA = 0.0566  # Schraudolph mean-bias correction
SCH_B16 = (127.0 - SCH_SIGMA - TEMP_INV * LOG2E) * 2.0**7
SCH_K16 = 2.0**-9

F32 = mybir.dt.float32
BF16 = mybir.dt.bfloat16
I16 = mybir.dt.int16

GROUPS = [(0, 2048), (2048, 4096), (4096, 5120)]
PIECE_W = 1024  # max exp piece width (psum tile, 2 banks)


def schedule(n_sc: int = 22):
    """Static per-core schedule (identical on all cores).

    Returns (groups, nslot): groups is a list of
    dict(G0, G1, batches=[dict(k, pieces=[(p0, p1, eng, slot)],
    cols=[(a, b, start, stop)])]).
    """
    slot = 0
    groups = []
    pieces_flat = []
    for G0, G1 in GROUPS:
        batches = []
        for k in range(8):
            cs, ce = max(k * 128, G0), min(k * 128 + BAND * 128, G1)
            if cs >= ce:
                continue
            pieces = []
            p = cs
            while p < ce:
                pe_ = min(p + PIECE_W, ce)
                pieces.append([p, pe_, None, slot])
                pieces_flat.append(pieces[-1])
                slot += 1
                p = pe_
            # colsum range: strip cols minus diag chunk minus antipode chunk
            c0, c1 = max(k * 128 + 128, cs), min(k * 128 + 4096, ce)
            cols = []
            if c0 < c1:
                bounds = {c0, c1}
                bounds.update(b for b in range(0, LCOLS + 1, 512) if c0 < b < c1)
                bounds.update(
                    pp for pc in pieces for pp in pc[:2] if c0 < pp < c1
                )
                bs = sorted(bounds)
                cols = [[a, b, False, False] for a, b in zip(bs[:-1], bs[1:])]
            batches.append(dict(k=k, cs=cs, ce=ce, pieces=pieces, cols=cols))
        groups.append(dict(G0=G0, G1=G1, batches=batches))

    # engine assignment (Bresenham): n_sc of the pieces on ScalarE
    total = len(pieces_flat)
    s_used = 0
    for i, pc in enumerate(pieces_flat):
        if s_used * total < n_sc * (i + 1) and s_used < n_sc:
            pc[2] = "S"
            s_used += 1
        else:
            pc[2] = "D"

    # start/stop flags per (group, psum bank of colacc)
    for g in groups:
        first_seen, last_mm = {}, {}
        for batch in g["batches"]:
            for cm in batch["cols"]:
                bank = (cm[0] - g["G0"]) // 512
                if bank not in first_seen:
                    cm[2] = True
                    first_seen[bank] = True
                last_mm[bank] = cm
        for cm in last_mm.values():
            cm[3] = True
    return groups, slot


def build_nc(loop_k: int = 1, n_sc: int = 22):
    groups, nslot = schedule(n_sc)
    nc = bacc.Bacc("TRN2", target_bir_lowering=False, debug=False)

    xt_d = nc.dram_tensor("xt", [128, LCOLS], BF16, kind="ExternalInput")
    rowp_d = nc.dram_tensor("rowp", [128, nslot], F32, kind="ExternalOutput")
    colp_d = nc.dram_tensor("colp", [1, LCOLS], F32, kind="ExternalOutput")

    with tile.TileContext(nc) as tc:
        with (
            tc.tile_pool(name="persist", bufs=1) as persist,
            tc.tile_pool(name="expool", bufs=6) as expool,
            tc.tile_pool(name="mpsum", bufs=2, space="PSUM") as mpsum,
            tc.tile_pool(name="colpsum", bufs=1, space="PSUM") as colpsum,
        ):
            xt = persist.tile([128, LCOLS], BF16, tag="xt")
            ones = persist.tile([128, 1], BF16, tag="ones")
            bneg10 = persist.tile([128, 1], F32, tag="bneg10")
            rowp = persist.tile([128, nslot], F32, tag="rowp")
            colsb = persist.tile([1, LCOLS], F32, tag="colsb")

            nc.vector.memset(ones[:], 1.0)
            nc.vector.memset(bneg10[:], -TEMP_INV)

            def body():
                for i in range(5):
                    nc.sync.dma_start(
                        xt[:, i * 1024 : (i + 1) * 1024],
                        xt_d[:, i * 1024 : (i + 1) * 1024],
                    )

                for g in groups:
                    G0, G1 = g["G0"], g["G1"]
                    colacc = colpsum.tile([128, 2048], F32, tag="colacc")
                    ext_of = {}  # piece p0 -> (tile, p0, bf16_view)

                    def emit_colsums(batch):
                        for a, b, st, sp in batch["cols"]:
                            # find the piece containing [a, b)
                            for p0, p1, eng, slot_ in batch["pieces"]:
                                if p0 <= a and b <= p1:
                                    break
                            ext, q0, view = ext_of[p0]
                            nc.tensor.matmul(
                                colacc[0:1, a - G0 : b - G0],
                                ones[:],
                                view[:, a - q0 : b - q0],
                                start=st,
                                stop=sp,
                                skip_group_check=True,
                            )

                    pending = None
                    for batch in g["batches"]:
                        k = batch["k"]
                        lhsT = xt[:, k * 128 : (k + 1) * 128]
                        for p0, p1, eng, slot_ in batch["pieces"]:
                            w = p1 - p0
                            ps = mpsum.tile([128, PIECE_W], F32, tag="ps")
                            for a in range(0, w, 512):
                                b = min(a + 512, w)
                                nc.tensor.matmul(
                                    ps[:, a:b],
                                    lhsT,
                                    xt[:, p0 + a : p0 + b],
                                    start=True,
                                    stop=True,
                                )
                            if eng == "S":
                                ext = expool.tile(
                                    [128, PIECE_W], BF16, tag="extS", name="extS"
                                )
                                nc.scalar.activation(
                                    ext[:, :w],
                                    ps[:, :w],
                                    mybir.ActivationFunctionType.Exp,
                                    bias=bneg10[:],
                                    scale=ACT_SCALE,
                                    accum_out=rowp[:, slot_ : slot_ + 1],
                                )
                                ext_of[p0] = (ext, p0, ext)
                            else:
                                ext = expool.tile(
                                    [128, PIECE_W], I16, tag="extD", name="extD"
                                )
                                nc.vector.tensor_scalar(
                                    out=ext[:, :w],
                                    in0=ps[:, :w],
                                    scalar1=SCH_K16,
                                    scalar2=SCH_B16,
                                    op0=mybir.AluOpType.mult,
                                    op1=mybir.AluOpType.add,
                                )
                                nc.vector.tensor_reduce(
                                    out=rowp[:, slot_ : slot_ + 1],
                                    in_=ext[:, :w].bitcast(BF16),
                                    axis=mybir.AxisListType.X,
                                    op=mybir.AluOpType.add,
                                )
                                ext_of[p0] = (ext, p0, ext.bitcast(BF16))
                        if pending is not None:
                            emit_colsums(pending)
                        pending = batch
                    emit_colsums(pending)

                    nc.scalar.copy(colsb[0:1, G0:G1], colacc[0:1, : G1 - G0])
                    nc.sync.dma_start(colp_d[0:1, G0:G1], colsb[0:1, G0:G1])

                nc.sync.dma_start(rowp_d[:], rowp[:])

            if loop_k == 1:
                body()
            else:
                with tc.For_i(0, loop_k, 1):
                    body()

    nc.compile()
    return nc


def prepare_inputs(embeddings: np.ndarray, labels: np.ndarray):
    """Host prep: normalize+scale rows to bf16; per-core rotated transposed
    views (only local columns [0, 5120)).  Returns (in_maps, aux) where aux
    carries everything the host combine step needs."""
    import ml_dtypes

    x = np.asarray(embeddings, dtype=np.float64)
    rn = 1.0 / np.maximum(np.sqrt((x * x).sum(axis=1)), 1e-12)
    e = x * rn[:, None]
    xn = (e * math.sqrt(C16)).astype(ml_dtypes.bfloat16)

    in_maps = []
    for c in range(NCORES):
        loc = np.roll(xn, -c * 1024, axis=0)
        xt_host = np.ascontiguousarray(loc.T[:, :LCOLS])
        in_maps.append({"xt": xt_host})

    lab = np.asarray(labels).astype(np.int64).ravel()
    xnf = xn.astype(np.float64)
    gdiag = (xnf * xnf).sum(axis=1) / C16
    diag_term = np.exp(TEMP_INV * gdiag - TEMP_INV)
    counts = np.bincount(lab, minlength=NCLASS).astype(np.float64)
    S = np.zeros((NCLASS, D))
    np.add.at(S, lab, e)
    d = np.einsum("ij,ij->i", e, S[lab])
    Ci = counts[lab] - 1.0
    aux = dict(diag_term=diag_term, d=d, Ci=Ci)
    return in_maps, aux


def combine(results, aux) -> np.ndarray:
    """Host combine: per-core rowp/colp partials -> loss."""
    groups, nslot = schedule()
    # strip of each slot
    slot_strip = np.empty(nslot, dtype=np.int64)
    for g in groups:
        for batch in g["batches"]:
            for p0, p1, eng, slot_ in batch["pieces"]:
                slot_strip[slot_] = batch["k"]

    rowsum = np.zeros(N)
    colsum = np.zeros(N)
    ridx = np.arange(1024)
    cidx = np.arange(CSPAN_LO, CSPAN_HI)
    for c in range(NCORES):
        rowp = np.asarray(results[c]["rowp"], dtype=np.float64)  # [128, nslot]
        colp = np.asarray(results[c]["colp"], dtype=np.float64).ravel()
        lrow = np.zeros(1024)
        for k in range(8):
            sl = np.where(slot_strip == k)[0]
            lrow[k * 128 : (k + 1) * 128] = rowp[:, sl].sum(axis=1)
        rowsum[(ridx + c * 1024) % N] += lrow
        colsum[(cidx + c * 1024) % N] += colp[CSPAN_LO:CSPAN_HI]

    sumexp = rowsum + colsum - aux["diag_term"]
    logz = TEMP_INV + np.log(sumexp)
    row_loss = aux["Ci"] * logz - TEMP_INV * (aux["d"] - 1.0)
    return np.asarray(row_loss.sum() / N, dtype=np.float32)


_NC_CACHE = {}


def kernel(embeddings: np.ndarray, labels: np.ndarray) -> np.ndarray:
    from concourse.bass_utils import run_bass_kernel_spmd

    nc = _NC_CACHE.get("nc")
    if nc is None:
        nc = _NC_CACHE["nc"] = build_nc(loop_k=1)
    in_maps, aux = prepare_inputs(embeddings, labels)
    res = run_bass_kernel_spmd(nc, in_maps, list(range(NCORES)))
    return combine(res.results, aux)
